# revision 36
# baseline (speedup 1.0000x reference)
"""VQ codebook pairwise squared-euclidean distances on 8 trn2 NeuronCores.

out[n, u] = ||x_n||^2 + ||w_u||^2 - 2 * x_n . w_u
  inputs: [16384, 1024] f32, w: [4096, 1024] f32 -> out [16384, 4096] f32

Strategy: data-parallel shard of N across 8 cores (2048 rows each), W
replicated. Per core: fp8(e4m3) GEMM on the tensor engine in DoubleRow
perf mode (fp32 PSUM accum; w pre-scaled by 64 into e4m3's normal
range). The device computes ONLY the scaled cross term (-2/64 * psum),
TRANSPOSED ([U, NS] tiles: w rows are the stationary operand, x the
moving one), and stores it as f16; the host transposes back and adds
the exact rank-1 terms ||x||^2 + ||w||^2 in f32 (they dominate the
magnitude, so fp8/f16 error stays ~1e-4 relative).

Why w-stationary: the first matmul then needs only one 128 KB k-slice
of w0 and one of x-chunk0 (~9.5 us after kernel start) instead of two
full 512 KB tiles (~14 us): the early DMA rings only run ~100 GB/s
per ring until ~15 us (8-core HBM contention), so the whole early
phase rides the supply curve k-slice by k-slice with no idle bridge.
Consumption order (u, then x-chunk, then u-stripe, kk innermost for
PSUM accumulation) keeps every matmul's inputs one small DMA ahead.

Other structure: ~4 warm-up matmuls on a memset tile cover the PE_HAM
clock-gate ramp; single-op VectorE epilogue (scaled PSUM -> f16, no
ScalarE ACT table so its DMA queue starts early); stores alternate
both HWDGE rings; wt2 rides the gpsimd SWDGE queue (~53 GB/s third
path); later w tiles prefetch from inside the u-loop so input
submissions never starve the store queue; final tiles compute as
N=256 halves to shrink the end-of-kernel serial tail.
"""

import sys

import ml_dtypes
import numpy as np

if "/opt/trn_rl_repo" not in sys.path:
    sys.path.insert(0, "/opt/trn_rl_repo")

N, D, U = 16384, 1024, 4096
NCORES = 8
NS = N // NCORES  # 2048 rows per core
P = 128
KK = D // 256  # 4 DoubleRow super k-tiles (256 contraction each)
MT = NS // P  # 16 m-tiles per core
UT = U // 512  # 8 u-tiles of 512 cols
MC = NS // 512  # 4 x-chunks of 512 rows (the moving operand)
ST = 4  # u-stripes of 128 within a u-tile (the stationary operand)
WSCALE = 64.0  # w pre-scale into e4m3 normal range (power of 2: exact)
NDUMMY = 4  # HAM warm-up matmuls before real data lands

_cache = {}


def _build():
    import concourse.bacc as bacc
    import concourse.mybir as mybir
    import concourse.tile as tile

    dt = mybir.dt
    DR = mybir.MatmulPerfMode.DoubleRow

    nc = bacc.Bacc("TRN2", debug=False, target_bir_lowering=False)
    # Early-critical tiles (w u-tile 0, x chunks 0-1) are stored kk-major so
    # each kk-slice is one contiguous 128 KB DMA; the rest load whole-tile.
    wth_d = nc.dram_tensor("wth", [KK, P, 2, 512], dt.float8e4, kind="ExternalInput")
    wtw_d = nc.dram_tensor("wtw", [UT - 1, P, KK, 2, 512], dt.float8e4, kind="ExternalInput")
    xth_d = nc.dram_tensor("xth", [2, KK, P, 2, 512], dt.float8e4, kind="ExternalInput")
    xtw_d = nc.dram_tensor("xtw", [MC - 2, P, KK, 2, 512], dt.float8e4, kind="ExternalInput")
    # Transposed cross term: rows = units, cols = this core's x rows.
    out_d = nc.dram_tensor("out", [U, NS], dt.float16, kind="ExternalOutput")

    with tile.TileContext(nc) as tc:
        with (
            tc.tile_pool(name="const", bufs=1) as cpool,
            tc.tile_pool(name="psum", bufs=7, space="PSUM") as psum_pool,
            tc.tile_pool(name="psumd", bufs=1, space="PSUM") as psumd_pool,
            tc.tile_pool(name="outp", bufs=20) as out_pool,
        ):
            # Warm-up fodder: a zeroed fp8 tile the PE chews on while the
            # first input DMAs stream in, so the HAM clock gate is ramping
            # before the first real matmul.
            zt = cpool.tile([P, 2, 640], dt.float8e4, tag="zt")
            nc.gpsimd.memset(zt[:], 0)
            psd = psumd_pool.tile([P, 512], dt.float32, tag="psd")
            for _ in range(NDUMMY):
                nc.tensor.matmul(
                    psd[:],
                    zt[:, :, 0:128],
                    zt[:, :, 128:640],
                    start=True,
                    stop=True,
                    perf_mode=DR,
                )

            xt_sb = {}
            wt_sb = {}

            def alloc_xt(c):
                t = cpool.tile([P, KK, 2, 512], dt.float8e4, tag=f"xt_{c}")
                xt_sb[c] = t
                return t

            def alloc_wt(u):
                t = cpool.tile([P, KK, 2, 512], dt.float8e4, tag=f"wt_{u}")
                wt_sb[u] = t
                return t

            # w0 kk-slices on the sync ring || x-chunk0 kk-slices on the
            # scalar ring; then x-chunk1 slices (sync) / w1 (scalar) /
            # x-chunk2-3; wt2 rides the gpsimd SWDGE queue; w3..w7 prefetch
            # from inside the u-loop.
            tw = alloc_wt(0)
            for kk in range(KK):
                nc.sync.dma_start(tw[:, kk], wth_d[kk])
            tx = alloc_xt(0)
            for kk in range(KK):
                nc.scalar.dma_start(tx[:, kk], xth_d[0, kk])
            tx1 = alloc_xt(1)
            for kk in range(KK):
                nc.sync.dma_start(tx1[:, kk], xth_d[1, kk])
            t = alloc_wt(2)
            nc.gpsimd.dma_start(t[:], wtw_d[1])
            t = alloc_wt(1)
            nc.scalar.dma_start(t[:], wtw_d[0])
            t = alloc_xt(2)
            nc.scalar.dma_start(t[:], xtw_d[0])
            t = alloc_xt(3)
            nc.sync.dma_start(t[:], xtw_d[1])

            def epilogue(u, s, c, ps, cols=512, c_off=0, flip=None):
                # Single-op epilogue on VectorE: scaled PSUM -> f16 SBUF (the
                # rank-1 bias terms are added host-side; ScalarE stays
                # activation-free so no ACT_TABLE_LOAD delays its DMA queue).
                ot = out_pool.tile([P, cols], dt.float16, tag=f"ot{cols}")
                nc.vector.tensor_scalar_mul(ot[:], ps[:], -2.0 / WSCALE)
                # Stores alternate rings (one ring ~ the f16 production rate).
                r0 = c * 512 + c_off
                dst = out_d[u * 512 + s * P : u * 512 + (s + 1) * P, r0 : r0 + cols]
                par = (u * MT + c * ST + s) % 2 if flip is None else flip
                eng = nc.sync if par == 0 else nc.scalar
                eng.dma_start(dst, ot[:])

            ntile = 0
            for u in range(UT):
                if 3 <= u + 2 < UT:
                    # Prefetch two u-passes ahead (~27 us of slack), engine
                    # alternating, interleaved with this pass's store subs.
                    t = alloc_wt(u + 2)
                    eng = nc.sync if u % 2 == 0 else nc.scalar
                    eng.dma_start(t[:], wtw_d[u + 1])
                for c in range(MC):
                    for s in range(ST):
                        ntile += 1
                        if ntile > UT * MC * ST - 2:
                            # Final tiles compute as two N=256 halves so the
                            # last epilogue+store overlaps the last matmuls.
                            for h in range(2):
                                psh = psum_pool.tile([P, 256], dt.float32, tag="ps")
                                for kk in range(KK):
                                    nc.tensor.matmul(
                                        psh[:],
                                        wt_sb[u][:, kk, :, s * P : (s + 1) * P],
                                        xt_sb[c][:, kk, :, h * 256 : (h + 1) * 256],
                                        start=(kk == 0),
                                        stop=(kk == KK - 1),
                                        perf_mode=DR,
                                    )
                                epilogue(u, s, c, psh, cols=256, c_off=h * 256, flip=h)
                            continue
                        ps = psum_pool.tile([P, 512], dt.float32, tag="ps")
                        for kk in range(KK):
                            nc.tensor.matmul(
                                ps[:],
                                wt_sb[u][:, kk, :, s * P : (s + 1) * P],
                                xt_sb[c][:, kk, :, :],
                                start=(kk == 0),
                                stop=(kk == KK - 1),
                                perf_mode=DR,
                            )
                        epilogue(u, s, c, ps)
    nc.compile()
    return nc


def _get_nc():
    if "nc" not in _cache:
        _cache["nc"] = _build()
    return _cache["nc"]


def _prep_inputs(inputs, w):
    f8 = ml_dtypes.float8_e4m3
    x = np.ascontiguousarray(np.asarray(inputs, dtype=np.float32))
    wf = np.ascontiguousarray(np.asarray(w, dtype=np.float32))

    # [u, p, kk, i, c]: element = w[u*512 + c, kk*256 + i*128 + p] * WSCALE
    wt = (wf * WSCALE).astype(f8).reshape(UT, 512, KK, 2, P).transpose(0, 4, 2, 3, 1)
    wth = np.ascontiguousarray(wt[0].transpose(1, 0, 2, 3))  # [kk, p, i, c]
    wtw = np.ascontiguousarray(wt[1:])
    w_sq = (wf.astype(np.float64) ** 2).sum(-1).astype(np.float32)  # [U]
    x_sq = (x.astype(np.float64) ** 2).sum(-1).astype(np.float32)  # [N]

    in_maps = []
    for core in range(NCORES):
        xs = x[core * NS : (core + 1) * NS]
        # [mc, p, kk, i, c]: element = x[n = mc*512 + col, d = kk*256 + i*128 + p]
        xt = xs.astype(f8).reshape(MC, 512, KK, 2, P).transpose(0, 4, 2, 3, 1)
        xth = np.ascontiguousarray(xt[0:2].transpose(0, 2, 1, 3, 4))  # kk-major
        xtw = np.ascontiguousarray(xt[2:])
        in_maps.append({"wth": wth, "wtw": wtw, "xth": xth, "xtw": xtw})
    return in_maps, x_sq, w_sq


def _post(crossT_f16, x_sq, w_sq):
    """crossT: [NCORES*U, NS] f16 device output (already scaled by -2).
    Transpose per core, add ||x||^2 + ||w||^2 in f32 on host."""
    per_core = crossT_f16.reshape(NCORES, U, NS)
    out = np.empty((N, U), dtype=np.float32)
    for core in range(NCORES):
        blk = out[core * NS : (core + 1) * NS]
        np.copyto(blk, per_core[core].T)
        blk += x_sq[core * NS : (core + 1) * NS, None]
        blk += w_sq[None, :]
    return out


def run(inputs, w, trace=False, **trace_kwargs):
    """Run on hardware via concourse; returns (out, BassKernelResults)."""
    from concourse.bass_utils import run_bass_kernel_spmd

    nc = _get_nc()
    in_maps, x_sq, w_sq = _prep_inputs(inputs, w)
    res = run_bass_kernel_spmd(
        nc, in_maps, list(range(NCORES)), trace=trace, **trace_kwargs
    )
    crossT = np.concatenate([r["out"] for r in res.results], axis=0)
    return _post(crossT, x_sq, w_sq), res


def _get_runner():
    """Cached jitted SPMD executable (mirrors bass2jax.run_bass_via_pjrt's
    multi-core branch) so repeat kernel() calls skip recompilation."""
    if "runner" in _cache:
        return _cache["runner"]
    import jax
    from concourse import bass2jax as b2j
    from concourse import mybir
    from jax.experimental.shard_map import shard_map
    from jax.sharding import Mesh, PartitionSpec

    nc = _get_nc()
    b2j.install_neuronx_cc_hook()
    partition_name = nc.partition_id_tensor.name if nc.partition_id_tensor else None
    in_names, out_names, out_avals, zero_shapes = [], [], [], []
    for alloc in nc.m.functions[0].allocations:
        if not isinstance(alloc, mybir.MemoryLocationSet):
            continue
        name = alloc.memorylocations[0].name
        if alloc.kind == "ExternalInput":
            if name != partition_name:
                in_names.append(name)
        elif alloc.kind == "ExternalOutput":
            out_names.append(name)
            shape, dtype = tuple(alloc.tensor_shape), mybir.dt.np(alloc.dtype)
            out_avals.append(jax.core.ShapedArray(shape, dtype))
            zero_shapes.append((shape, dtype))
    n_params, n_outs = len(in_names), len(out_names)
    all_in_names = in_names + out_names + ([partition_name] if partition_name else [])

    def _body(*args):
        operands = list(args)
        if partition_name is not None:
            operands.append(b2j.partition_id_tensor())
        return tuple(
            b2j._bass_exec_p.bind(
                *operands,
                out_avals=tuple(out_avals),
                in_names=tuple(all_in_names),
                out_names=tuple(out_names),
                lowering_input_output_aliases=(),
                sim_require_finite=True,
                sim_require_nnan=True,
                nc=nc,
            )
        )

    devices = jax.devices()[:NCORES]
    mesh = Mesh(np.asarray(devices), ("core",))
    sharded = jax.jit(
        shard_map(
            _body,
            mesh=mesh,
            in_specs=(PartitionSpec("core"),) * (n_params + n_outs),
            out_specs=(PartitionSpec("core"),) * n_outs,
            check_rep=False,
        ),
        donate_argnums=tuple(range(n_params, n_params + n_outs)),
        keep_unused=True,
    )

    # Donated output backing store, created device-side (the kernel writes
    # every element, so the zeros never cross the host<->device tunnel).
    import jax.numpy as jnp
    from jax.sharding import NamedSharding

    sharding = NamedSharding(mesh, PartitionSpec("core"))

    def zeros_maker(shape, dtype):
        return jax.jit(
            lambda: jnp.zeros((NCORES * shape[0], *shape[1:]), dtype),
            out_shardings=sharding,
        )

    makers = [zeros_maker(s, dt) for s, dt in zero_shapes]
    _cache["runner"] = (sharded, in_names, out_names, makers)
    return _cache["runner"]


def kernel(inputs, w):
    try:
        sharded, in_names, out_names, makers = _get_runner()
        in_maps, x_sq, w_sq = _prep_inputs(inputs, w)
        concat_in = [
            np.concatenate([m[name] for m in in_maps], axis=0) for name in in_names
        ]
        concat_zeros = [mk() for mk in makers]
        out_arrs = sharded(*concat_in, *concat_zeros)
        crossT = np.asarray(out_arrs[out_names.index("out")])
        return _post(crossT, x_sq, w_sq)
    except Exception:
        # Fallback: stock concourse SPMD runner (recompiles per call but has
        # no dependence on bass2jax internals).
        out, _ = run(inputs, w)
        return out


# revision 37
# speedup vs baseline: 1.0217x; 1.0217x over previous
"""VQ codebook pairwise squared-euclidean distances on 8 trn2 NeuronCores.

out[n, u] = ||x_n||^2 + ||w_u||^2 - 2 * x_n . w_u
  inputs: [16384, 1024] f32, w: [4096, 1024] f32 -> out [16384, 4096] f32

Strategy: data-parallel shard of N across 8 cores (2048 rows each), W
replicated. Per core: fp8(e4m3) GEMM on the tensor engine in DoubleRow
perf mode (2 MACs/cell/cycle, fp32 PSUM accum; w pre-scaled by 64 into
e4m3's normal range). The device computes ONLY the scaled cross term
(-2/64 * psum) and stores it as f16; the exact rank-1 terms
||x||^2 + ||w||^2 are added on the host in f32 (they dominate the
magnitude, so fp8/f16 error stays ~1e-4 relative).

v2 changes vs the f32-epilogue baseline (133-140 us):
- f16 cross-term stores (16.8 MB vs 33.5 MB): no ring congestion, short
  drain tail, and store-completion semaphore recycling never blocks.
- single-op epilogue alternating ScalarE/VectorE (the f32 version's
  ACT+ADD pair made ScalarE a near-critical 126 us resource); all store
  submissions moved to the Sync queue engine so epilogue engines never
  block on store semaphores (the old kernel lost 4 us + a HAM re-throttle
  to exactly that at ~35 us).
- input DMAs split across both HWDGE rings in consumption order, first
  tiles sliced by k so the first matmul issues ~4-5 us earlier.
- ~18 warm-up matmuls on a memset tile during the input-DMA wait so the
  PE_HAM clock gate is at 8/8 (2.4 GHz) when real matmuls start.
- PSUM pool 4 -> 6 banks.
"""

import sys

import ml_dtypes
import numpy as np

if "/opt/trn_rl_repo" not in sys.path:
    sys.path.insert(0, "/opt/trn_rl_repo")

N, D, U = 16384, 1024, 4096
NCORES = 8
NS = N // NCORES  # 2048 rows per core
P = 128
KK = D // 256  # 4 DoubleRow super k-tiles (256 contraction each)
MT = NS // P  # 16 m-tiles per core
UT = U // 512  # 8 u-tiles of 512 cols
MC = 4  # m-tiles per xt load chunk (512 cols)
WSCALE = 64.0  # w pre-scale into e4m3 normal range (power of 2: exact)
NDUMMY = 16  # HAM warm-up matmuls before real data lands

_cache = {}


def _build():
    import concourse.bacc as bacc
    import concourse.mybir as mybir
    import concourse.tile as tile

    dt = mybir.dt
    AF = mybir.ActivationFunctionType
    DR = mybir.MatmulPerfMode.DoubleRow

    nc = bacc.Bacc("TRN2", debug=False, target_bir_lowering=False)
    # Host-pre-packed fp8 inputs: block b holds [p=128, kk, i, c] where the
    # contraction index is d = kk*256 + i*128 + p (DoubleRow packs pairs
    # (p, i) into one PE cell). Each block is one contiguous 512 KB region.
    # Whole-tile contiguous loads: each early DMA pays ~0.75 us of fixed
    # queue overhead on top of the transfer, so a few big DMAs beat many
    # k-slices (measured: 4x128 KB slices land at ~14 us, one 512 KB tile
    # at ~11 us from kernel start).
    xt_d = nc.dram_tensor("xt", [MT // MC, P, KK, 2, 512], dt.float8e4, kind="ExternalInput")
    wt_d = nc.dram_tensor("wt", [UT, P, KK, 2, 512], dt.float8e4, kind="ExternalInput")
    out_d = nc.dram_tensor("out", [NS, U], dt.float16, kind="ExternalOutput")

    with tile.TileContext(nc) as tc:
        with (
            tc.tile_pool(name="const", bufs=1) as cpool,
            tc.tile_pool(name="psum", bufs=7, space="PSUM") as psum_pool,
            tc.tile_pool(name="psumd", bufs=1, space="PSUM") as psumd_pool,
            tc.tile_pool(name="outp", bufs=20) as out_pool,
        ):
            # Warm-up fodder: a zeroed fp8 tile the PE can chew on while the
            # first input DMAs stream in, so the HAM clock gate reaches 8/8
            # before the first real matmul.
            zt = cpool.tile([P, 2, 640], dt.float8e4, tag="zt")
            nc.gpsimd.memset(zt[:], 0)
            psd = psumd_pool.tile([P, 512], dt.float32, tag="psd")
            for _ in range(NDUMMY):
                nc.tensor.matmul(
                    psd[:],
                    zt[:, :, 0:128],
                    zt[:, :, 128:640],
                    start=True,
                    stop=True,
                    perf_mode=DR,
                )

            xt_sb = {}
            wt_sb = {}

            def alloc_xt(mc):
                t = cpool.tile([P, KK, 2, 512], dt.float8e4, tag=f"xt_{mc}")
                xt_sb[mc] = t
                return t

            def alloc_wt(u):
                t = cpool.tile([P, KK, 2, 512], dt.float8e4, tag=f"wt_{u}")
                wt_sb[u] = t
                return t

            # Input DMAs split across both HWDGE rings in consumption order.
            # The first x/w tiles are sliced by kk (128 KB pieces) so the
            # first accumulation group can start as soon as its k-slices
            # land rather than waiting for full 512 KB tiles.
            # wt0 streams on the sync ring while xt0 streams on the scalar
            # ring; the next-needed tiles follow in consumption order.
            # wt2..wt7 are prefetched from inside the u-loop so the
            # sync/scalar engines finish their input submissions early and
            # store submissions are never starved (out_pool backpressure
            # stalled the PE otherwise).
            t = alloc_wt(0)
            nc.sync.dma_start(t[:], wt_d[0])
            t = alloc_xt(0)
            nc.scalar.dma_start(t[:], xt_d[0])
            # xt2 rides the gpsimd SWDGE queue (~53 GB/s, but a genuinely
            # parallel third path that frees early ring bytes); it has ~12 us
            # of slack vs its first use.
            t = alloc_xt(2)
            nc.gpsimd.dma_start(t[:], xt_d[2])
            t = alloc_xt(1)
            nc.sync.dma_start(t[:], xt_d[1])
            t = alloc_wt(1)
            nc.scalar.dma_start(t[:], wt_d[1])
            t = alloc_xt(3)
            nc.sync.dma_start(t[:], xt_d[3])

            def epilogue(u, m, ps):
                # Single-op epilogue on VectorE: scaled PSUM -> f16 SBUF (the
                # rank-1 bias terms are added host-side). ScalarE is kept
                # activation-free so no ACT_TABLE_LOAD delays its DMA queue.
                ot = out_pool.tile([P, 512], dt.float16, tag="ot")
                nc.vector.tensor_scalar_mul(ot[:], ps[:], -2.0 / WSCALE)
                # Stores alternate rings: one ring tops out ~150 GB/s, which
                # is about the f16 store production rate — split so neither
                # ring saturates and the drain tail stays short.
                dst = out_d[m * P : (m + 1) * P, u * 512 : (u + 1) * 512]
                eng = nc.sync if (u * MT + m) % 2 == 0 else nc.scalar
                eng.dma_start(dst, ot[:])

            # u=0, m=0..3: kk-major over 4 concurrent PSUM accumulation
            # groups, so every arriving 128 KB k-slice immediately unlocks
            # 4 matmuls — the PE rides the input stream's slow ramp without
            # ever idling long enough to re-throttle the clock gate.
            ps_head = []
            for _ in range(MC):
                ps_h = psum_pool.tile([P, 512], dt.float32, tag="ps")
                ps_head.append(ps_h)
            for kk in range(KK):
                for mo in range(MC):
                    nc.tensor.matmul(
                        ps_head[mo][:],
                        xt_sb[0][:, kk, :, mo * P : (mo + 1) * P],
                        wt_sb[0][:, kk, :, :],
                        start=(kk == 0),
                        stop=(kk == KK - 1),
                        perf_mode=DR,
                    )
            for mo in range(MC):
                epilogue(0, mo, ps_head[mo])

            for u in range(UT):
                if u + 2 < UT:
                    # Prefetch two u-passes ahead (~27 us of slack), engine
                    # alternating, interleaved with this pass's store subs.
                    t = alloc_wt(u + 2)
                    eng = nc.sync if u % 2 == 0 else nc.scalar
                    eng.dma_start(t[:], wt_d[u + 2])
                for m in range(MC if u == 0 else 0, MT):
                    mc, mo = divmod(m, MC)
                    if u == UT - 1 and m >= MT - 2:
                        # Final tiles compute as two N=256 halves so the last
                        # epilogue+store overlaps the last matmuls and the
                        # end-of-kernel serial tail shrinks.
                        for h in range(2):
                            psh = psum_pool.tile([P, 256], dt.float32, tag="ps")
                            for kk in range(KK):
                                nc.tensor.matmul(
                                    psh[:],
                                    xt_sb[mc][:, kk, :, mo * P : (mo + 1) * P],
                                    wt_sb[u][:, kk, :, h * 256 : (h + 1) * 256],
                                    start=(kk == 0),
                                    stop=(kk == KK - 1),
                                    perf_mode=DR,
                                )
                            ot = out_pool.tile([P, 256], dt.float16, tag="ot2")
                            nc.vector.tensor_scalar_mul(ot[:], psh[:], -2.0 / WSCALE)
                            eng = nc.sync if h == 0 else nc.scalar
                            eng.dma_start(
                                out_d[
                                    m * P : (m + 1) * P,
                                    u * 512 + h * 256 : u * 512 + (h + 1) * 256,
                                ],
                                ot[:],
                            )
                        continue
                    ps = psum_pool.tile([P, 512], dt.float32, tag="ps")
                    for kk in range(KK):
                        nc.tensor.matmul(
                            ps[:],
                            xt_sb[mc][:, kk, :, mo * P : (mo + 1) * P],
                            wt_sb[u][:, kk, :, :],
                            start=(kk == 0),
                            stop=(kk == KK - 1),
                            perf_mode=DR,
                        )
                    epilogue(u, m, ps)
    nc.compile()
    return nc


def _get_nc():
    if "nc" not in _cache:
        _cache["nc"] = _build()
    return _cache["nc"]


def _prep_inputs(inputs, w):
    f8 = ml_dtypes.float8_e4m3
    x = np.ascontiguousarray(np.asarray(inputs, dtype=np.float32))
    wf = np.ascontiguousarray(np.asarray(w, dtype=np.float32))

    # [u, p, kk, i, c]: element = w[u*512 + c, kk*256 + i*128 + p] * WSCALE
    wt = np.ascontiguousarray(
        (wf * WSCALE).astype(f8).reshape(UT, 512, KK, 2, P).transpose(0, 4, 2, 3, 1)
    )
    w_sq = (wf.astype(np.float64) ** 2).sum(-1).astype(np.float32)  # [U]
    x_sq = (x.astype(np.float64) ** 2).sum(-1).astype(np.float32)  # [N]

    in_maps = []
    for c in range(NCORES):
        xs = x[c * NS : (c + 1) * NS]
        # [mc, p, kk, i, c]: element = x[n = mc*512 + col, d = kk*256 + i*128 + p]
        xt = np.ascontiguousarray(
            xs.astype(f8).reshape(MT // MC, 512, KK, 2, P).transpose(0, 4, 2, 3, 1)
        )
        in_maps.append({"xt": xt, "wt": wt})
    return in_maps, x_sq, w_sq


def _post(cross_f16, x_sq, w_sq):
    """cross (already scaled by -2) + ||x||^2 + ||w||^2, in f32 on host."""
    out = cross_f16.astype(np.float32)
    out += x_sq[:, None]
    out += w_sq[None, :]
    return np.ascontiguousarray(out)


def run(inputs, w, trace=False, **trace_kwargs):
    """Run on hardware via concourse; returns (out, BassKernelResults)."""
    from concourse.bass_utils import run_bass_kernel_spmd

    nc = _get_nc()
    in_maps, x_sq, w_sq = _prep_inputs(inputs, w)
    res = run_bass_kernel_spmd(
        nc, in_maps, list(range(NCORES)), trace=trace, **trace_kwargs
    )
    cross = np.concatenate([r["out"] for r in res.results], axis=0)
    return _post(cross, x_sq, w_sq), res


def _get_runner():
    """Cached jitted SPMD executable (mirrors bass2jax.run_bass_via_pjrt's
    multi-core branch) so repeat kernel() calls skip recompilation."""
    if "runner" in _cache:
        return _cache["runner"]
    import jax
    from concourse import bass2jax as b2j
    from concourse import mybir
    from jax.experimental.shard_map import shard_map
    from jax.sharding import Mesh, PartitionSpec

    nc = _get_nc()
    b2j.install_neuronx_cc_hook()
    partition_name = nc.partition_id_tensor.name if nc.partition_id_tensor else None
    in_names, out_names, out_avals, zero_shapes = [], [], [], []
    for alloc in nc.m.functions[0].allocations:
        if not isinstance(alloc, mybir.MemoryLocationSet):
            continue
        name = alloc.memorylocations[0].name
        if alloc.kind == "ExternalInput":
            if name != partition_name:
                in_names.append(name)
        elif alloc.kind == "ExternalOutput":
            out_names.append(name)
            shape, dtype = tuple(alloc.tensor_shape), mybir.dt.np(alloc.dtype)
            out_avals.append(jax.core.ShapedArray(shape, dtype))
            zero_shapes.append((shape, dtype))
    n_params, n_outs = len(in_names), len(out_names)
    all_in_names = in_names + out_names + ([partition_name] if partition_name else [])

    def _body(*args):
        operands = list(args)
        if partition_name is not None:
            operands.append(b2j.partition_id_tensor())
        return tuple(
            b2j._bass_exec_p.bind(
                *operands,
                out_avals=tuple(out_avals),
                in_names=tuple(all_in_names),
                out_names=tuple(out_names),
                lowering_input_output_aliases=(),
                sim_require_finite=True,
                sim_require_nnan=True,
                nc=nc,
            )
        )

    devices = jax.devices()[:NCORES]
    mesh = Mesh(np.asarray(devices), ("core",))
    sharded = jax.jit(
        shard_map(
            _body,
            mesh=mesh,
            in_specs=(PartitionSpec("core"),) * (n_params + n_outs),
            out_specs=(PartitionSpec("core"),) * n_outs,
            check_rep=False,
        ),
        donate_argnums=tuple(range(n_params, n_params + n_outs)),
        keep_unused=True,
    )

    # Donated output backing store, created device-side (the kernel writes
    # every element, so the zeros never cross the host<->device tunnel).
    import jax.numpy as jnp
    from jax.sharding import NamedSharding

    sharding = NamedSharding(mesh, PartitionSpec("core"))

    def zeros_maker(shape, dtype):
        return jax.jit(
            lambda: jnp.zeros((NCORES * shape[0], *shape[1:]), dtype),
            out_shardings=sharding,
        )

    makers = [zeros_maker(s, dt) for s, dt in zero_shapes]
    _cache["runner"] = (sharded, in_names, out_names, makers)
    return _cache["runner"]


def kernel(inputs, w):
    try:
        sharded, in_names, out_names, makers = _get_runner()
        in_maps, x_sq, w_sq = _prep_inputs(inputs, w)
        concat_in = [
            np.concatenate([m[name] for m in in_maps], axis=0) for name in in_names
        ]
        concat_zeros = [mk() for mk in makers]
        out_arrs = sharded(*concat_in, *concat_zeros)
        cross = np.asarray(out_arrs[out_names.index("out")]).reshape(N, U)
        return _post(cross, x_sq, w_sq)
    except Exception:
        # Fallback: stock concourse SPMD runner (recompiles per call but has
        # no dependence on bass2jax internals).
        out, _ = run(inputs, w)
        return out


# revision 39
# speedup vs baseline: 1.0297x; 1.0079x over previous
"""VQ codebook pairwise squared-euclidean distances on 8 trn2 NeuronCores.

out[n, u] = ||x_n||^2 + ||w_u||^2 - 2 * x_n . w_u
  inputs: [16384, 1024] f32, w: [4096, 1024] f32 -> out [16384, 4096] f32

Strategy: data-parallel shard of N across 8 cores (2048 rows each), W
replicated. Per core: fp8(e4m3) GEMM on the tensor engine in DoubleRow
perf mode (2 MACs/cell/cycle, fp32 PSUM accum; w pre-scaled by 64 into
e4m3's normal range). The device computes ONLY the scaled cross term
(-2/64 * psum) and stores it as f16; the exact rank-1 terms
||x||^2 + ||w||^2 are added on the host in f32 (they dominate the
magnitude, so fp8/f16 error stays ~1e-4 relative).

v2 changes vs the f32-epilogue baseline (133-140 us):
- f16 cross-term stores (16.8 MB vs 33.5 MB): no ring congestion, short
  drain tail, and store-completion semaphore recycling never blocks.
- single-op epilogue alternating ScalarE/VectorE (the f32 version's
  ACT+ADD pair made ScalarE a near-critical 126 us resource); all store
  submissions moved to the Sync queue engine so epilogue engines never
  block on store semaphores (the old kernel lost 4 us + a HAM re-throttle
  to exactly that at ~35 us).
- input DMAs split across both HWDGE rings in consumption order, first
  tiles sliced by k so the first matmul issues ~4-5 us earlier.
- ~18 warm-up matmuls on a memset tile during the input-DMA wait so the
  PE_HAM clock gate is at 8/8 (2.4 GHz) when real matmuls start.
- PSUM pool 4 -> 6 banks.
"""

import sys

import ml_dtypes
import numpy as np

if "/opt/trn_rl_repo" not in sys.path:
    sys.path.insert(0, "/opt/trn_rl_repo")

N, D, U = 16384, 1024, 4096
NCORES = 8
NS = N // NCORES  # 2048 rows per core
P = 128
KK = D // 256  # 4 DoubleRow super k-tiles (256 contraction each)
MT = NS // P  # 16 m-tiles per core
UT = U // 512  # 8 u-tiles of 512 cols
MC = 4  # m-tiles per xt load chunk (512 cols)
WSCALE = 64.0  # w pre-scale into e4m3 normal range (power of 2: exact)
NDUMMY = 14  # HAM warm-up matmuls before real data lands

_cache = {}


def _build():
    import concourse.bacc as bacc
    import concourse.mybir as mybir
    import concourse.tile as tile

    dt = mybir.dt
    AF = mybir.ActivationFunctionType
    DR = mybir.MatmulPerfMode.DoubleRow

    nc = bacc.Bacc("TRN2", debug=False, target_bir_lowering=False)
    # Host-pre-packed fp8 inputs: block b holds [p=128, kk, i, c] where the
    # contraction index is d = kk*256 + i*128 + p (DoubleRow packs pairs
    # (p, i) into one PE cell). Each block is one contiguous 512 KB region.
    # Whole-tile contiguous loads: each early DMA pays ~0.75 us of fixed
    # queue overhead on top of the transfer, so a few big DMAs beat many
    # k-slices (measured: 4x128 KB slices land at ~14 us, one 512 KB tile
    # at ~11 us from kernel start).
    xt_d = nc.dram_tensor("xt", [MT // MC, P, KK, 2, 512], dt.float8e4, kind="ExternalInput")
    wt_d = nc.dram_tensor("wt", [UT, P, KK, 2, 512], dt.float8e4, kind="ExternalInput")
    out_d = nc.dram_tensor("out", [NS, U], dt.float16, kind="ExternalOutput")

    with tile.TileContext(nc) as tc:
        with (
            tc.tile_pool(name="const", bufs=1) as cpool,
            tc.tile_pool(name="psum", bufs=7, space="PSUM") as psum_pool,
            tc.tile_pool(name="psumd", bufs=1, space="PSUM") as psumd_pool,
            tc.tile_pool(name="outp", bufs=20) as out_pool,
        ):
            # Warm-up fodder: a zeroed fp8 tile the PE can chew on while the
            # first input DMAs stream in, so the HAM clock gate reaches 8/8
            # before the first real matmul.
            zt = cpool.tile([P, 2, 640], dt.float8e4, tag="zt")
            nc.gpsimd.memset(zt[:], 0)
            psd = psumd_pool.tile([P, 512], dt.float32, tag="psd")
            for _ in range(NDUMMY):
                nc.tensor.matmul(
                    psd[:],
                    zt[:, :, 0:128],
                    zt[:, :, 128:640],
                    start=True,
                    stop=True,
                    perf_mode=DR,
                )

            xt_sb = {}
            wt_sb = {}

            def alloc_xt(mc):
                t = cpool.tile([P, KK, 2, 512], dt.float8e4, tag=f"xt_{mc}")
                xt_sb[mc] = t
                return t

            def alloc_wt(u):
                t = cpool.tile([P, KK, 2, 512], dt.float8e4, tag=f"wt_{u}")
                wt_sb[u] = t
                return t

            # Input DMAs split across both HWDGE rings in consumption order.
            # The first x/w tiles are sliced by kk (128 KB pieces) so the
            # first accumulation group can start as soon as its k-slices
            # land rather than waiting for full 512 KB tiles.
            # wt0 streams on the sync ring while xt0 streams on the scalar
            # ring; the next-needed tiles follow in consumption order.
            # wt2..wt7 are prefetched from inside the u-loop so the
            # sync/scalar engines finish their input submissions early and
            # store submissions are never starved (out_pool backpressure
            # stalled the PE otherwise).
            t = alloc_wt(0)
            nc.sync.dma_start(t[:], wt_d[0])
            t = alloc_xt(0)
            nc.scalar.dma_start(t[:], xt_d[0])
            # xt2 rides the gpsimd SWDGE queue (~53 GB/s, but a genuinely
            # parallel third path that frees early ring bytes); it has ~12 us
            # of slack vs its first use.
            t = alloc_xt(2)
            nc.gpsimd.dma_start(t[:], xt_d[2])
            t = alloc_xt(1)
            nc.sync.dma_start(t[:], xt_d[1])
            t = alloc_wt(1)
            nc.scalar.dma_start(t[:], wt_d[1])
            t = alloc_xt(3)
            nc.sync.dma_start(t[:], xt_d[3])

            def epilogue(u, m, ps):
                # Single-op epilogue on VectorE: scaled PSUM -> f16 SBUF (the
                # rank-1 bias terms are added host-side). ScalarE is kept
                # activation-free so no ACT_TABLE_LOAD delays its DMA queue.
                ot = out_pool.tile([P, 512], dt.float16, tag="ot")
                nc.vector.tensor_scalar_mul(ot[:], ps[:], -2.0 / WSCALE)
                # Stores alternate rings: one ring tops out ~150 GB/s, which
                # is about the f16 store production rate — split so neither
                # ring saturates and the drain tail stays short.
                dst = out_d[m * P : (m + 1) * P, u * 512 : (u + 1) * 512]
                eng = nc.sync if (u * MT + m) % 2 == 0 else nc.scalar
                eng.dma_start(dst, ot[:])

            # u=0, m=0..3: kk-major over 4 concurrent PSUM accumulation
            # groups, so every arriving 128 KB k-slice immediately unlocks
            # 4 matmuls — the PE rides the input stream's slow ramp without
            # ever idling long enough to re-throttle the clock gate.
            ps_head = []
            for _ in range(MC):
                ps_h = psum_pool.tile([P, 512], dt.float32, tag="ps")
                ps_head.append(ps_h)
            for kk in range(KK):
                for mo in range(MC):
                    nc.tensor.matmul(
                        ps_head[mo][:],
                        xt_sb[0][:, kk, :, mo * P : (mo + 1) * P],
                        wt_sb[0][:, kk, :, :],
                        start=(kk == 0),
                        stop=(kk == KK - 1),
                        perf_mode=DR,
                    )
            for mo in range(MC):
                epilogue(0, mo, ps_head[mo])

            for u in range(UT):
                if u + 2 < UT:
                    # Prefetch two u-passes ahead (~27 us of slack), engine
                    # alternating, interleaved with this pass's store subs.
                    t = alloc_wt(u + 2)
                    eng = nc.sync if u % 2 == 0 else nc.scalar
                    eng.dma_start(t[:], wt_d[u + 2])
                for m in range(MC if u == 0 else 0, MT):
                    mc, mo = divmod(m, MC)
                    ps = psum_pool.tile([P, 512], dt.float32, tag="ps")
                    for kk in range(KK):
                        nc.tensor.matmul(
                            ps[:],
                            xt_sb[mc][:, kk, :, mo * P : (mo + 1) * P],
                            wt_sb[u][:, kk, :, :],
                            start=(kk == 0),
                            stop=(kk == KK - 1),
                            perf_mode=DR,
                        )
                    epilogue(u, m, ps)
    nc.compile()
    return nc


def _get_nc():
    if "nc" not in _cache:
        _cache["nc"] = _build()
    return _cache["nc"]


def _prep_inputs(inputs, w):
    f8 = ml_dtypes.float8_e4m3
    x = np.ascontiguousarray(np.asarray(inputs, dtype=np.float32))
    wf = np.ascontiguousarray(np.asarray(w, dtype=np.float32))

    # [u, p, kk, i, c]: element = w[u*512 + c, kk*256 + i*128 + p] * WSCALE
    wt = np.ascontiguousarray(
        (wf * WSCALE).astype(f8).reshape(UT, 512, KK, 2, P).transpose(0, 4, 2, 3, 1)
    )
    w_sq = (wf.astype(np.float64) ** 2).sum(-1).astype(np.float32)  # [U]
    x_sq = (x.astype(np.float64) ** 2).sum(-1).astype(np.float32)  # [N]

    in_maps = []
    for c in range(NCORES):
        xs = x[c * NS : (c + 1) * NS]
        # [mc, p, kk, i, c]: element = x[n = mc*512 + col, d = kk*256 + i*128 + p]
        xt = np.ascontiguousarray(
            xs.astype(f8).reshape(MT // MC, 512, KK, 2, P).transpose(0, 4, 2, 3, 1)
        )
        in_maps.append({"xt": xt, "wt": wt})
    return in_maps, x_sq, w_sq


def _post(cross_f16, x_sq, w_sq):
    """cross (already scaled by -2) + ||x||^2 + ||w||^2, in f32 on host."""
    out = cross_f16.astype(np.float32)
    out += x_sq[:, None]
    out += w_sq[None, :]
    return np.ascontiguousarray(out)


def run(inputs, w, trace=False, **trace_kwargs):
    """Run on hardware via concourse; returns (out, BassKernelResults)."""
    from concourse.bass_utils import run_bass_kernel_spmd

    nc = _get_nc()
    in_maps, x_sq, w_sq = _prep_inputs(inputs, w)
    res = run_bass_kernel_spmd(
        nc, in_maps, list(range(NCORES)), trace=trace, **trace_kwargs
    )
    cross = np.concatenate([r["out"] for r in res.results], axis=0)
    return _post(cross, x_sq, w_sq), res


def _get_runner():
    """Cached jitted SPMD executable (mirrors bass2jax.run_bass_via_pjrt's
    multi-core branch) so repeat kernel() calls skip recompilation."""
    if "runner" in _cache:
        return _cache["runner"]
    import jax
    from concourse import bass2jax as b2j
    from concourse import mybir
    from jax.experimental.shard_map import shard_map
    from jax.sharding import Mesh, PartitionSpec

    nc = _get_nc()
    b2j.install_neuronx_cc_hook()
    partition_name = nc.partition_id_tensor.name if nc.partition_id_tensor else None
    in_names, out_names, out_avals, zero_shapes = [], [], [], []
    for alloc in nc.m.functions[0].allocations:
        if not isinstance(alloc, mybir.MemoryLocationSet):
            continue
        name = alloc.memorylocations[0].name
        if alloc.kind == "ExternalInput":
            if name != partition_name:
                in_names.append(name)
        elif alloc.kind == "ExternalOutput":
            out_names.append(name)
            shape, dtype = tuple(alloc.tensor_shape), mybir.dt.np(alloc.dtype)
            out_avals.append(jax.core.ShapedArray(shape, dtype))
            zero_shapes.append((shape, dtype))
    n_params, n_outs = len(in_names), len(out_names)
    all_in_names = in_names + out_names + ([partition_name] if partition_name else [])

    def _body(*args):
        operands = list(args)
        if partition_name is not None:
            operands.append(b2j.partition_id_tensor())
        return tuple(
            b2j._bass_exec_p.bind(
                *operands,
                out_avals=tuple(out_avals),
                in_names=tuple(all_in_names),
                out_names=tuple(out_names),
                lowering_input_output_aliases=(),
                sim_require_finite=True,
                sim_require_nnan=True,
                nc=nc,
            )
        )

    devices = jax.devices()[:NCORES]
    mesh = Mesh(np.asarray(devices), ("core",))
    sharded = jax.jit(
        shard_map(
            _body,
            mesh=mesh,
            in_specs=(PartitionSpec("core"),) * (n_params + n_outs),
            out_specs=(PartitionSpec("core"),) * n_outs,
            check_rep=False,
        ),
        donate_argnums=tuple(range(n_params, n_params + n_outs)),
        keep_unused=True,
    )

    # Donated output backing store, created device-side (the kernel writes
    # every element, so the zeros never cross the host<->device tunnel).
    import jax.numpy as jnp
    from jax.sharding import NamedSharding

    sharding = NamedSharding(mesh, PartitionSpec("core"))

    def zeros_maker(shape, dtype):
        return jax.jit(
            lambda: jnp.zeros((NCORES * shape[0], *shape[1:]), dtype),
            out_shardings=sharding,
        )

    makers = [zeros_maker(s, dt) for s, dt in zero_shapes]
    _cache["runner"] = (sharded, in_names, out_names, makers)
    return _cache["runner"]


def kernel(inputs, w):
    try:
        sharded, in_names, out_names, makers = _get_runner()
        in_maps, x_sq, w_sq = _prep_inputs(inputs, w)
        concat_in = [
            np.concatenate([m[name] for m in in_maps], axis=0) for name in in_names
        ]
        concat_zeros = [mk() for mk in makers]
        out_arrs = sharded(*concat_in, *concat_zeros)
        cross = np.asarray(out_arrs[out_names.index("out")]).reshape(N, U)
        return _post(cross, x_sq, w_sq)
    except Exception:
        # Fallback: stock concourse SPMD runner (recompiles per call but has
        # no dependence on bass2jax internals).
        out, _ = run(inputs, w)
        return out
